# revision 10
# baseline (speedup 1.0000x reference)
"""Trainium2 Bass kernel for nn_Attention_50500225466997.

Computation (per batch): qkv = BN(conv1x1(x)); 4-head attention over L=1024
(DK=32, DH=64); out = attn + BN(dwconv3x3(v)); y = BN(conv1x1(out)).

Strategy (v2 — fp8 DoubleRow attention):
  - Data-parallel over batch: 16 batches -> 8 NeuronCores, 2 per core.
  - Scores: fp8e4m3 Q/K in a folded [d16, 2, m] layout -> DoubleRow matmuls
    (0.5 cyc/row).  Q/K are evacuated PSUM->bf16 (bias applied) and folded
    to fp8 by gpsimd cast-DMAs.
  - Softmax: exp shifted by C=0.5 (softmax-invariant) to keep E in fp8
    range.  ACT computes 3/4 of exp tiles -> fp8e4m3; DVE computes 1/4 via
    a Schraudolph int8 bit-trick -> fp8e5m2 (single tensor_scalar).
  - O-matmul: fp8 DoubleRow over lt-pairs with ones-augmented vT lhsT
    ([O_e;Z_e] / [Z_o;O_o] banks); normalize = 2 stream_shuffles + fast
    reciprocal + 2 multiplies on DVE.
  - Depthwise 3x3 via 9 bf16 diagonal-matrix matmuls accumulating in PSUM;
    added into out2 by DVE tensor_tensor.
  - All evacuations carry their BN bias on the ACT/DVE op (no extra work).
  - Cross-batch software pipelining as in v1.
"""

import numpy as np

import concourse.bass as bass
import concourse.mybir as mybir
import concourse.tile as tile
from concourse import bacc
from concourse.bass_utils import run_bass_kernel_spmd

F32 = mybir.dt.float32
BF16 = mybir.dt.bfloat16
FP8 = mybir.dt.float8e4
FP8E5 = mybir.dt.float8e5
I8 = mybir.dt.int8
AF = mybir.ActivationFunctionType
OP = mybir.AluOpType
PM = mybir.MatmulPerfMode

B, CH, HH, WW = 16, 256, 32, 32
L = HH * WW                   # 1024
NH, DK, DH = 4, 32, 64
CQKV = CH + DK * NH * 2       # 512
SCALE = DK ** (-0.5)
NCORES = 8
BL = B // NCORES              # batches per core

EXP_SHIFT = 0.5
SCH_A = 4.0 / np.log(2.0)                       # e5m2: 4/ln2
SCH_C = 15 * 4 + 0.5 - SCH_A * EXP_SHIFT - 0.2  # bias + trunc->round + tune

ID32 = list(range(32))

DEBUG = False

# which lt-pairs (of 4) are computed by DVE-schraudolph (fp8e5) vs ACT (fp8e4)
DVE_PAIRS = (3,)


def build_bass():
    nc = bacc.Bacc("TRN2", target_bir_lowering=False, debug=False)

    x_d = nc.dram_tensor("x", [BL, CH, L], BF16, kind="ExternalInput")
    wqkvT_d = nc.dram_tensor("wqkvT", [128, 2, CQKV], BF16, kind="ExternalInput")
    bqkv_d = nc.dram_tensor("bqkv", [128, 4], F32, kind="ExternalInput")
    wpwT_d = nc.dram_tensor("wpwT", [128, 2, CH], BF16, kind="ExternalInput")
    bpw_d = nc.dram_tensor("bpw", [128, 2], F32, kind="ExternalInput")
    diag_d = nc.dram_tensor("diag", [128, 18, 128], BF16, kind="ExternalInput")
    id2_d = nc.dram_tensor("id2", [128, 64], BF16, kind="ExternalInput")
    out_d = nc.dram_tensor("out", [BL, CH, L], BF16, kind="ExternalOutput")
    if DEBUG:
        dbg_qev = nc.dram_tensor("dbg_qev", [128, L], F32, kind="ExternalOutput")
        dbg_qa8 = nc.dram_tensor("dbg_qa8", [128, 2, L], F32, kind="ExternalOutput")
        dbg_ka8 = nc.dram_tensor("dbg_ka8", [128, 2, L], F32, kind="ExternalOutput")
        dbg_vf = nc.dram_tensor("dbg_vf", [128, L], F32, kind="ExternalOutput")
        dbg_vt = nc.dram_tensor("dbg_vt", [128, 8, 128], F32, kind="ExternalOutput")
        dbg_et = nc.dram_tensor("dbg_et", [128, 2, 1024], F32, kind="ExternalOutput")
        dbg_o2 = nc.dram_tensor("dbg_o2", [128, L], F32, kind="ExternalOutput")

    with tile.TileContext(nc) as tc, nc.allow_low_precision(reason="fp8"):
        with (
            tc.tile_pool(name="consts", bufs=1) as consts,
            tc.tile_pool(name="xin", bufs=4) as xin,
            tc.tile_pool(name="qk8", bufs=4) as qk8p,
            tc.tile_pool(name="qkev", bufs=4) as qkevp,
            tc.tile_pool(name="vt", bufs=1) as vtp,
            tc.tile_pool(name="et4", bufs=4) as et4p,
            tc.tile_pool(name="et5", bufs=2) as et5p,
            tc.tile_pool(name="o2", bufs=4) as o2p,
            tc.tile_pool(name="small", bufs=6) as smallp,
            tc.tile_pool(name="pad", bufs=4) as padp,
            tc.tile_pool(name="outp", bufs=4) as outp,
            tc.tile_pool(name="psc", bufs=4, space="PSUM") as psc,
            tc.tile_pool(name="pO", bufs=2, space="PSUM") as pOp,
            tc.tile_pool(name="pwork", bufs=2, space="PSUM") as pwork,
        ):
            # ---------------- constants ----------------
            wqkvT = consts.tile([128, 2, CQKV], BF16)
            bqkv = consts.tile([128, 4], F32)
            nc.sync.dma_start(bqkv, bqkv_d.ap())
            nc.sync.dma_start(wqkvT[:, :, 0:256], wqkvT_d.ap()[:, :, 0:256])
            id2 = consts.tile([128, 64], BF16)
            wpwT = consts.tile([128, 2, CH], BF16)
            bpw = consts.tile([128, 2], F32)
            diag = consts.tile([128, 18, 128], BF16)
            ebias = consts.tile([128, 1], F32)
            nc.vector.memset(ebias, -EXP_SHIFT)

            # prefetch all batches' x
            Xall = []
            for b in range(BL):
                Xb = []
                for ct in range(2):
                    xt = xin.tile([128, L], BF16, name=f"x_b{b}c{ct}", tag="x")
                    Xb.append(xt)
                if b == 0:
                    qeng = [nc.scalar, nc.gpsimd]
                    for q in range(2):
                        for ct in range(2):
                            qs = slice(256 * q, 256 * q + 256)
                            qeng[ct].dma_start(
                                Xb[ct][:, qs],
                                x_d.ap()[b, 128 * ct : 128 * ct + 128, qs],
                            )
                    for ct in range(2):
                        hs = slice(512, 1024)
                        qeng[ct].dma_start(
                            Xb[ct][:, hs], x_d.ap()[b, 128 * ct : 128 * ct + 128, hs]
                        )
                    nc.gpsimd.dma_start(id2, id2_d.ap())
                    nc.gpsimd.dma_start(
                        wqkvT[:, :, 256:512], wqkvT_d.ap()[:, :, 256:512]
                    )
                else:
                    for ct in range(2):
                        nc.sync.dma_start(
                            Xb[ct], x_d.ap()[b, 128 * ct : 128 * ct + 128, :]
                        )
                Xall.append(Xb)

            # persistent vt tiles (fp8): ones-halves preset once.
            # even heads: [vT | 1] -> bank [O_e; Z_e]; odd: [1 | vT] -> [Z_o; O_o]
            Vt = []
            for h in range(NH):
                par = h % 2
                vt_h = vtp.tile([128, 8, 128], FP8, name=f"vt{h}", tag=f"vt{h}")
                nc.vector.memset(vt_h[:, :, 64 - 64 * par : 128 - 64 * par], 1.0)
                Vt.append(vt_h)

            # persistent padded-v images (bf16): border zeroed once
            Pads = []
            for b in range(BL):
                pb = []
                for ct in range(2):
                    padt = padp.tile(
                        [128, 34, 34], BF16, name=f"pad{b}{ct}", tag=f"pad{b}{ct}"
                    )
                    nc.vector.memset(padt, 0.0)
                    pb.append(padt)
                Pads.append(pb)

            st = [{} for _ in range(BL)]

            def init_state(b):
                st[b].update(
                    Qa8=qk8p.tile([128, 2, L], FP8, name=f"Qa8_{b}", tag="Qa8"),
                    Ka8=qk8p.tile([128, 2, L], FP8, name=f"Ka8_{b}", tag="Ka8"),
                    Qev=qkevp.tile([128, L], BF16, name=f"Qev_{b}", tag="Qev"),
                    Kev=qkevp.tile([128, L], BF16, name=f"Kev_{b}", tag="Kev"),
                    Vf=[
                        qkevp.tile([128, L], BF16, name=f"Vf_{b}{ct}", tag=f"Vf{ct}")
                        for ct in range(2)
                    ],
                    out2=[
                        o2p.tile([128, L], BF16, name=f"o2_{b}{ct}", tag="o2")
                        for ct in range(2)
                    ],
                )

            def qkv_group(b, ot, mt, evac):
                """One (ot, mt) qkv matmul group + evacuation.
                evac: 'act' or 'dve' (engine for the PSUM->SBUF bias evac)."""
                X = Xall[b]
                Qev, Kev, Vf = st[b]["Qev"], st[b]["Kev"], st[b]["Vf"]
                cw = 256 if (b == 0 and mt == 0) else 512
                for off in range(512 * mt, 512 * mt + 512, cw):
                    ms = slice(off, off + cw)
                    pq = pwork.tile([128, cw], F32, name=f"pq{b}{ot}{off}", tag="w")
                    for kt in range(2):
                        nc.tensor.matmul(
                            pq,
                            wqkvT[:, kt, 128 * ot : 128 * ot + 128],
                            X[kt][:, ms],
                            start=(kt == 0),
                            stop=(kt == 1),
                        )
                    dst = [Qev, Kev, Vf[0], Vf[1]][ot]
                    if evac == "act":
                        nc.scalar.activation(
                            dst[:, ms], pq, AF.Identity, bias=bqkv[:, ot : ot + 1]
                        )
                    else:
                        nc.vector.tensor_scalar(
                            dst[:, ms], pq, bqkv[:, ot : ot + 1], None, OP.add
                        )
                    if ot >= 2:
                        ct = ot - 2
                        nr = cw // 32
                        nc.sync.dma_start(
                            Pads[b][ct][
                                :, 1 + off // 32 : 1 + off // 32 + nr, 1:33
                            ],
                            dst[:, ms].rearrange("p (a c) -> p a c", a=nr),
                        )

            def fold_qk(b):
                # evac rows are ordered 32h + 2d' + t (t innermost): one
                # cast-DMA per head folds [32, L] -> [16, 2, L] into fp8.
                Qa8, Ka8 = st[b]["Qa8"], st[b]["Ka8"]
                Qev, Kev = st[b]["Qev"], st[b]["Kev"]
                for h in range(NH):
                    nc.gpsimd.dma_start(
                        Qa8[32 * h : 32 * h + 16, :, :], Qev[32 * h : 32 * h + 32, :]
                    )
                    nc.gpsimd.dma_start(
                        Ka8[32 * h : 32 * h + 16, :, :], Kev[32 * h : 32 * h + 32, :]
                    )

            def transposes(b, h, evac):
                ct, lo = h // 2, (h % 2) * 64
                par = h % 2
                Vf = st[b]["Vf"]
                pv = pwork.tile([128, 512], BF16, name=f"pv{b}{h}", tag="w")
                for c8 in range(8):
                    nc.tensor.transpose(
                        pv[:, 64 * c8 : 64 * c8 + 64],
                        Vf[ct][lo : lo + 64, 128 * c8 : 128 * c8 + 128],
                        id2[lo : lo + 64, :],
                    )
                dst = Vt[h][:, :, 64 * par : 64 * par + 64]
                pvv = pv.rearrange("p (c d) -> p c d", c=8)
                if evac == "act":
                    nc.scalar.activation(dst, pvv, AF.Identity)
                else:
                    nc.vector.tensor_copy(dst, pvv)

            def dw_taps(b, ct, mt):
                padt = Pads[b][ct]
                dwp = pwork.tile([128, 512], F32, name=f"dw{b}{ct}{mt}", tag="w")
                for tap in range(9):
                    dy, dx = tap // 3, tap % 3
                    r0 = 16 * mt + dy
                    nc.tensor.matmul(
                        dwp,
                        diag[:, 9 * ct + tap, :],
                        padt[:, r0 : r0 + 16, dx : dx + 32],
                        start=(tap == 0),
                        stop=(tap == 8),
                    )
                return dwp

            def dw_add(b, ct, mt, dwp, fine=False):
                out2 = st[b]["out2"]
                steps = ((0, 256), (256, 256)) if fine else ((0, 512),)
                for off, cw in steps:
                    os_ = slice(512 * mt + off, 512 * mt + off + cw)
                    nc.vector.tensor_tensor(
                        out=out2[ct][:, os_],
                        in0=dwp[:, off : off + cw],
                        in1=out2[ct][:, os_],
                        op=OP.add,
                    )

            def pw_unit(b, mt, ot, evac, fine=False):
                out2 = st[b]["out2"]
                steps = ((0, 256), (256, 256)) if fine else ((0, 512),)
                for off, cw in steps:
                    ms = slice(512 * mt + off, 512 * mt + off + cw)
                    pp = pwork.tile(
                        [128, cw], F32, name=f"pp{b}{mt}{ot}{off}", tag="w"
                    )
                    for kt in range(2):
                        nc.tensor.matmul(
                            pp,
                            wpwT[:, kt, 128 * ot : 128 * ot + 128],
                            out2[kt][:, ms],
                            start=(kt == 0),
                            stop=(kt == 1),
                        )
                    osb = outp.tile(
                        [128, cw], BF16, name=f"os{b}{mt}{ot}{off}", tag="os"
                    )
                    if evac == "act":
                        nc.scalar.activation(
                            osb, pp, AF.Identity, bias=bpw[:, ot : ot + 1]
                        )
                    else:
                        nc.vector.tensor_scalar(
                            osb, pp, bpw[:, ot : ot + 1], None, OP.add
                        )
                    [nc.sync, nc.gpsimd][ot].dma_start(
                        out_d.ap()[b, 128 * ot : 128 * ot + 128, ms], osb
                    )

            def attn_unit(b, hp, mt, fillers=(), post=(), fine=False,
                          dve_pairs=DVE_PAIRS):
                """One (hp, mt) attention unit.  Software-pipelined: the
                scores+exp of pair p4+1 are emitted BEFORE pair p4's filler
                and O-matmuls, so the exp stream stays dense.  The LAST
                filler must be the unit's dw_taps (its PSUM tile is released
                only by the post-norm dw_add)."""
                Qa8, Ka8, out2 = st[b]["Qa8"], st[b]["Ka8"], st[b]["out2"]
                ms = slice(512 * mt, 512 * mt + 512)
                pA = pOp.tile([128, 512], F32, name=f"pa{b}{hp}{mt}", tag="o")
                pB = pOp.tile([128, 512], F32, name=f"pb{b}{hp}{mt}", tag="o")
                banks = [pA, pB]
                fl = list(fillers)
                Ets = [None] * 4

                def scores_exp(p4):
                    use_dve = p4 in dve_pairs
                    Et = (et5p if use_dve else et4p).tile(
                        [128, 2, 1024],
                        FP8E5 if use_dve else FP8,
                        name=f"e{b}{hp}{mt}{p4}",
                        tag="e5" if use_dve else "e4",
                    )
                    for half in range(2):
                        lt = 2 * p4 + half
                        ls = slice(128 * lt, 128 * lt + 128)
                        for j in range(2):
                            h = 2 * hp + j
                            sc = psc.tile(
                                [128, 512], F32,
                                name=f"sc{b}{hp}{mt}{lt}{j}", tag="sc",
                            )
                            nc.tensor.matmul(
                                sc,
                                Ka8[32 * h : 32 * h + 16, :, ls],
                                Qa8[32 * h : 32 * h + 16, :, ms],
                                start=True,
                                stop=True,
                                perf_mode=PM.DoubleRow,
                                tile_position=(32 * h, 0),
                            )
                            dst = Et[:, half, 512 * j : 512 * j + 512]
                            if use_dve:
                                nc.vector.tensor_scalar(
                                    dst.bitcast(I8), sc,
                                    float(SCH_A), float(SCH_C),
                                    OP.mult, OP.add,
                                )
                            else:
                                nc.scalar.activation(
                                    dst, sc, AF.Exp, bias=ebias
                                )
                    Ets[p4] = Et

                scores_exp(0)
                for p4 in range(4):
                    if p4 < 3:
                        scores_exp(p4 + 1)
                        if fl and len(fl) > 1:
                            fl.pop(0)()
                    else:
                        for f in fl:
                            f()
                        fl = []
                    Et = Ets[p4]
                    for j in range(2):
                        h = 2 * hp + j
                        nc.tensor.matmul(
                            banks[j],
                            Vt[h][:, 2 * p4 : 2 * p4 + 2, :],
                            Et[:, :, 512 * j : 512 * j + 512],
                            start=(p4 == 0),
                            stop=(p4 == 3),
                            perf_mode=PM.DoubleRow,
                            skip_group_check=True,
                        )
                # normalize: ZA = [Z_e; Z_o] via shuffles, recip, 2 mults
                steps = ((0, 256), (256, 256)) if fine else ((0, 512),)
                for off, cw in steps:
                    cs = slice(off, off + cw)
                    os_ = slice(512 * mt + off, 512 * mt + off + cw)
                    ZA = smallp.tile(
                        [128, cw], F32, name=f"za{b}{hp}{mt}{off}", tag="za"
                    )
                    nc.vector.stream_shuffle(ZA[0:64, :], pA[64:128, cs], ID32)
                    nc.vector.stream_shuffle(ZA[64:128, :], pB[0:64, cs], ID32)
                    Rz = smallp.tile(
                        [128, cw], F32, name=f"rz{b}{hp}{mt}{off}", tag="rz"
                    )
                    nc.vector.reciprocal_approx_fast(out=Rz, in_=ZA)
                    nc.vector.tensor_tensor(
                        out=out2[hp][0:64, os_],
                        in0=pA[0:64, cs],
                        in1=Rz[0:64, :],
                        op=OP.mult,
                    )
                    nc.vector.tensor_tensor(
                        out=out2[hp][64:128, os_],
                        in0=pB[64:128, cs],
                        in1=Rz[64:128, :],
                        op=OP.mult,
                    )
                for p in post:
                    p()

            def make_dw(b, ct, mt, fine=False):
                state = {}

                def taps():
                    state["dwp"] = dw_taps(b, ct, mt)

                def add():
                    dw_add(b, ct, mt, state["dwp"], fine)

                return taps, add

            assert BL == 2
            init_state(0)
            init_state(1)

            # prologue: b0 K/Q only (evacs: K on ACT, Q on DVE), fold asap
            for mt in range(2):
                qkv_group(0, 1, mt, "act")
                qkv_group(0, 0, mt, "dve")
            fold_qk(0)
            nc.scalar.dma_start(wpwT, wpwT_d.ap())
            nc.gpsimd.dma_start(bpw, bpw_d.ap())
            nc.gpsimd.dma_start(diag, diag_d.ap())

            # b0 units.  V(0)/transposes(0) are woven in as early fillers
            # (evacs on DVE - ACT is running the exp stream); units whose
            # DVE is evac-heavy run all-ACT exp.
            t00, a00 = make_dw(0, 0, 0)
            attn_unit(0, 0, 0, fillers=[
                lambda: (qkv_group(0, 2, 0, "dve"), qkv_group(0, 2, 1, "dve"),
                         transposes(0, 0, "dve"), transposes(0, 1, "dve")),
                lambda: (qkv_group(0, 3, 0, "dve"), qkv_group(0, 3, 1, "dve")),
                t00,
            ], post=[a00], dve_pairs=())
            t01, a01 = make_dw(0, 0, 1)
            attn_unit(0, 0, 1, fillers=[
                lambda: (transposes(0, 2, "dve"), transposes(0, 3, "dve")),
                lambda: qkv_group(1, 1, 0, "dve"),
                t01,
            ], post=[a01], dve_pairs=())
            t10, a10 = make_dw(0, 1, 0)
            attn_unit(0, 1, 0, fillers=[
                lambda: qkv_group(1, 1, 1, "dve"),
                lambda: (qkv_group(1, 0, 0, "dve"), qkv_group(1, 0, 1, "dve"),
                         fold_qk(1)),
                t10,
            ], post=[a10])
            t11, a11 = make_dw(0, 1, 1)
            attn_unit(0, 1, 1, fillers=[
                lambda: (qkv_group(1, 2, 0, "dve"), qkv_group(1, 2, 1, "dve")),
                lambda: (qkv_group(1, 3, 0, "dve"), qkv_group(1, 3, 1, "dve")),
                t11,
            ], post=[a11])
            # batch-1 vT transposes: must follow ALL batch-0 O-matmuls (Vt
            # tiles are shared), so emit between u3 and u4.
            for h in range(NH):
                transposes(1, h, "dve")

            # b1 units; fillers: b0 pw, then b1 mt=0 pw inside u7
            s00, b00 = make_dw(1, 0, 0)
            attn_unit(1, 0, 0, fillers=[
                lambda: pw_unit(0, 0, 0, "act"),
                lambda: pw_unit(0, 0, 1, "dve"),
                s00,
            ], post=[b00])
            s01, b01 = make_dw(1, 0, 1)
            attn_unit(1, 0, 1, fillers=[
                lambda: pw_unit(0, 1, 0, "act"),
                lambda: pw_unit(0, 1, 1, "dve"),
                s01,
            ], post=[b01])
            s10, b10 = make_dw(1, 1, 0)
            attn_unit(1, 1, 0, fillers=[s10], post=[b10])
            s11, b11 = make_dw(1, 1, 1, fine=True)
            attn_unit(1, 1, 1, fillers=[
                lambda: pw_unit(1, 0, 0, "act"),
                lambda: pw_unit(1, 0, 1, "dve"),
                s11,
            ], post=[b11], fine=True)

            # tail: b1 mt=1 pw
            pw_unit(1, 1, 0, "act", fine=True)
            pw_unit(1, 1, 1, "dve", fine=True)

    nc.compile()
    return nc


def pack_inputs(w_qkv, s_qkv, b_qkv, w_dw, s_dw, b_dw, w_pw, s_pw, b_pw):
    """Host-side weight packing. Returns dict of constant arrays (shared by
    all cores)."""
    import ml_dtypes

    f32 = np.float32
    bf16_ = ml_dtypes.bfloat16
    Wq = (w_qkv[:, :, 0, 0] * s_qkv[:, None]).astype(np.float64)  # [512, 256]
    bq = b_qkv.astype(np.float64).copy()

    # output-channel permutation:
    #   Q rows (0-127):   m = 64t + 16h + d'  <- orig channel 128h + 16t + d'
    #   K rows (128-255): m = 64t + 16h + d'  <- orig channel 128h + 32 + 16t + d'
    #   V rows (256-511): ct-major [h-pair 64+64]
    perm = []
    for h in range(NH):
        for d in range(16):
            for t in range(2):
                perm += [h * 128 + 16 * t + d]                         # q
    for h in range(NH):
        for d in range(16):
            for t in range(2):
                perm += [h * 128 + 32 + 16 * t + d]                    # k
    for h in range(NH):
        perm += [h * 128 + 64 + d for d in range(64)]                  # v
    perm = np.array(perm)
    Wq = Wq[perm]
    bq = bq[perm]
    # fold attention scale into q
    Wq[0:128] *= SCALE
    bq[0:128] *= SCALE

    wqkvT = np.ascontiguousarray(
        Wq.T.reshape(2, 128, CQKV).transpose(1, 0, 2)
    ).astype(bf16_)  # [128, 2, 512]
    bqkv = np.ascontiguousarray(bq.reshape(4, 128).T).astype(f32)  # [128, 4]

    Wp = (w_pw[:, :, 0, 0] * s_pw[:, None]).astype(np.float64)     # [256, 256]
    bp = b_pw.astype(np.float64) + Wp @ b_dw.astype(np.float64)
    wpwT = np.ascontiguousarray(
        Wp.T.reshape(2, 128, CH).transpose(1, 0, 2)
    ).astype(bf16_)  # [128, 2, 256]
    bpw = np.ascontiguousarray(bp.reshape(2, 128).T).astype(f32)   # [128, 2]

    wd = (w_dw[:, 0] * s_dw[:, None, None]).astype(f32)            # [256, 3, 3]
    diag = np.zeros((128, 18, 128), f32)
    for ct in range(2):
        for tap in range(9):
            dy, dx = tap // 3, tap % 3
            idx = np.arange(128)
            diag[idx, 9 * ct + tap, idx] = wd[128 * ct + idx, dy, dx]

    id2 = np.tile(np.eye(64, dtype=f32), (2, 1))                   # [128, 64]

    return {
        "wqkvT": wqkvT,
        "bqkv": bqkv,
        "wpwT": wpwT,
        "bpw": bpw,
        "diag": diag.astype(bf16_),
        "id2": id2.astype(bf16_),
    }


_NC_CACHE = None


def _get_nc():
    global _NC_CACHE
    if _NC_CACHE is None:
        _NC_CACHE = build_bass()
    return _NC_CACHE


def run(inputs, trace=False):
    """Run the bass kernel on 8 cores. inputs = the reference input dict.
    Returns (full_output [16,256,32,32], BassKernelResults)."""
    import ml_dtypes

    x = np.ascontiguousarray(
        np.asarray(inputs["x"], dtype=np.float32).astype(ml_dtypes.bfloat16)
    ).reshape(B, CH, L)
    consts = pack_inputs(
        np.asarray(inputs["w_qkv"], np.float32),
        np.asarray(inputs["s_qkv"], np.float32),
        np.asarray(inputs["b_qkv"], np.float32),
        np.asarray(inputs["w_dw"], np.float32),
        np.asarray(inputs["s_dw"], np.float32),
        np.asarray(inputs["b_dw"], np.float32),
        np.asarray(inputs["w_pw"], np.float32),
        np.asarray(inputs["s_pw"], np.float32),
        np.asarray(inputs["b_pw"], np.float32),
    )
    in_maps = []
    for c in range(NCORES):
        m = dict(consts)
        m["x"] = np.ascontiguousarray(x[c * BL : (c + 1) * BL])
        in_maps.append(m)

    nc = _get_nc()
    res = run_bass_kernel_spmd(
        nc, in_maps, core_ids=list(range(NCORES)), trace=trace
    )
    out = np.concatenate(
        [r["out"].astype(np.float32) for r in res.results], axis=0
    )
    return out.reshape(B, CH, HH, WW), res


def kernel(**inputs) -> np.ndarray:
    out, _ = run(inputs, trace=False)
    return out


# revision 11
# speedup vs baseline: 1.3921x; 1.3921x over previous
"""Trainium2 Bass kernel for nn_Attention_50500225466997.

Computation (per batch): qkv = BN(conv1x1(x)); 4-head attention over L=1024
(DK=32, DH=64); out = attn + BN(dwconv3x3(v)); y = BN(conv1x1(out)).

Strategy (v4):
  - Data-parallel over batch: 16 batches -> 8 NeuronCores, 2 per core.
  - Scores in bf16 (K=32 matmuls, tile_position packing) - the fat score
    streams keep the PE dense (pstate at full clock).
  - Softmax: exp shifted by C=0.5 (softmax-invariant) to keep E in fp8
    range.  ACT computes most exp tiles -> fp8e4m3; DVE computes 1/4 via a
    Schraudolph int8 bit-trick -> fp8e5m2 (single tensor_scalar f32->int8).
  - O-matmul: fp8 DoubleRow over lt-pairs (0.5 cyc/row, 2 k-tiles per
    instruction) with ones-augmented fp8 vT lhsT ([O_e;Z_e] / [Z_o;O_o]
    banks); normalize = 2 stream_shuffles + fast reciprocal + 2 multiplies.
  - Depthwise 3x3 via 9 bf16 diagonal matmuls accumulating in PSUM; the
    padded image is filled from the flat V tile by a SBUF->SBUF DMA.
  - All evacuations carry their BN bias on the ACT/DVE op.
  - Software-pipelined emission: each attention (hp,mt) unit carries PE
    "filler" (dw taps, next-batch qkv, transposes, pw) between its exp
    emissions and O-matmuls so no engine starves.
"""

import numpy as np

import concourse.bass as bass
import concourse.mybir as mybir
import concourse.tile as tile
from concourse import bacc
from concourse.bass_utils import run_bass_kernel_spmd

F32 = mybir.dt.float32
BF16 = mybir.dt.bfloat16
FP8 = mybir.dt.float8e4
FP8E5 = mybir.dt.float8e5
I8 = mybir.dt.int8
AF = mybir.ActivationFunctionType
OP = mybir.AluOpType
PM = mybir.MatmulPerfMode

B, CH, HH, WW = 16, 256, 32, 32
L = HH * WW                   # 1024
NH, DK, DH = 4, 32, 64
CQKV = CH + DK * NH * 2       # 512
SCALE = DK ** (-0.5)
NCORES = 8
BL = B // NCORES              # batches per core

EXP_SHIFT = 0.5
SCH_A = 4.0 / np.log(2.0)                       # e5m2: 4/ln2
SCH_C = 15 * 4 + 0.5 - SCH_A * EXP_SHIFT - 0.2  # bias + trunc->round + tune

ID32 = list(range(32))

# which lt-pairs (of 4) are computed by DVE-schraudolph (fp8e5) vs ACT (fp8e4)
DVE_PAIRS = (3,)


def build_bass():
    nc = bacc.Bacc("TRN2", target_bir_lowering=False, debug=False)

    x_d = nc.dram_tensor("x", [BL, CH, L], BF16, kind="ExternalInput")
    wqkvT_d = nc.dram_tensor("wqkvT", [128, 2, CQKV], BF16, kind="ExternalInput")
    bqkv_d = nc.dram_tensor("bqkv", [128, 4], F32, kind="ExternalInput")
    wpwT_d = nc.dram_tensor("wpwT", [128, 2, CH], BF16, kind="ExternalInput")
    bpw_d = nc.dram_tensor("bpw", [128, 2], F32, kind="ExternalInput")
    diag_d = nc.dram_tensor("diag", [128, 18, 128], BF16, kind="ExternalInput")
    id2_d = nc.dram_tensor("id2", [128, 64], BF16, kind="ExternalInput")
    out_d = nc.dram_tensor("out", [BL, CH, L], BF16, kind="ExternalOutput")

    with tile.TileContext(nc) as tc, nc.allow_low_precision(reason="fp8"):
        with (
            tc.tile_pool(name="consts", bufs=1) as consts,
            tc.tile_pool(name="xin", bufs=4) as xin,
            tc.tile_pool(name="qk", bufs=4) as qkp,
            tc.tile_pool(name="vt", bufs=1) as vtp,
            tc.tile_pool(name="et4", bufs=4) as et4p,
            tc.tile_pool(name="et5", bufs=2) as et5p,
            tc.tile_pool(name="o2", bufs=4) as o2p,
            tc.tile_pool(name="small", bufs=6) as smallp,
            tc.tile_pool(name="pad", bufs=4) as padp,
            tc.tile_pool(name="outp", bufs=4) as outp,
            tc.tile_pool(name="psc", bufs=2, space="PSUM") as psc,
            tc.tile_pool(name="pO", bufs=2, space="PSUM") as pOp,
            tc.tile_pool(name="pwork", bufs=2, space="PSUM") as pwork,
        ):
            # ---------------- constants ----------------
            wqkvT = consts.tile([128, 2, CQKV], BF16)
            bqkv = consts.tile([128, 4], F32)
            nc.sync.dma_start(bqkv, bqkv_d.ap())
            nc.sync.dma_start(wqkvT[:, :, 0:256], wqkvT_d.ap()[:, :, 0:256])
            id2 = consts.tile([128, 64], BF16)
            wpwT = consts.tile([128, 2, CH], BF16)
            bpw = consts.tile([128, 2], F32)
            diag = consts.tile([128, 18, 128], BF16)
            ebias = consts.tile([128, 1], F32)
            nc.vector.memset(ebias, -EXP_SHIFT)

            # prefetch all batches' x
            Xall = []
            for b in range(BL):
                Xb = []
                for ct in range(2):
                    xt = xin.tile([128, L], BF16, name=f"x_b{b}c{ct}", tag="x")
                    Xb.append(xt)
                if b == 0:
                    qeng = [nc.scalar, nc.gpsimd]
                    for q in range(2):
                        for ct in range(2):
                            qs = slice(256 * q, 256 * q + 256)
                            qeng[ct].dma_start(
                                Xb[ct][:, qs],
                                x_d.ap()[b, 128 * ct : 128 * ct + 128, qs],
                            )
                    for ct in range(2):
                        hs = slice(512, 1024)
                        qeng[ct].dma_start(
                            Xb[ct][:, hs], x_d.ap()[b, 128 * ct : 128 * ct + 128, hs]
                        )
                    nc.gpsimd.dma_start(id2, id2_d.ap())
                    nc.gpsimd.dma_start(
                        wqkvT[:, :, 256:512], wqkvT_d.ap()[:, :, 256:512]
                    )
                else:
                    for ct in range(2):
                        nc.sync.dma_start(
                            Xb[ct], x_d.ap()[b, 128 * ct : 128 * ct + 128, :]
                        )
                Xall.append(Xb)

            # persistent vt tiles (fp8): ones-halves preset once.
            # even heads: [vT | 1] -> bank [O_e; Z_e]; odd: [1 | vT] -> [Z_o; O_o]
            Vt = []
            for h in range(NH):
                par = h % 2
                vt_h = vtp.tile([128, 8, 128], FP8, name=f"vt{h}", tag=f"vt{h}")
                nc.vector.memset(vt_h[:, :, 64 - 64 * par : 128 - 64 * par], 1.0)
                Vt.append(vt_h)

            # persistent padded-v images (bf16): border zeroed once
            Pads = []
            for b in range(BL):
                pb = []
                for ct in range(2):
                    padt = padp.tile(
                        [128, 34, 34], BF16, name=f"pad{b}{ct}", tag=f"pad{b}{ct}"
                    )
                    nc.vector.memset(padt, 0.0)
                    pb.append(padt)
                Pads.append(pb)

            st = [{} for _ in range(BL)]

            def init_state(b):
                st[b].update(
                    Qa=qkp.tile([128, L], BF16, name=f"Qa_{b}", tag="Qa"),
                    Ka=qkp.tile([128, L], BF16, name=f"Ka_{b}", tag="Ka"),
                    Vf=[
                        qkp.tile([128, L], BF16, name=f"Vf_{b}{ct}", tag=f"Vf{ct}")
                        for ct in range(2)
                    ],
                    out2=[
                        o2p.tile([128, L], BF16, name=f"o2_{b}{ct}", tag="o2")
                        for ct in range(2)
                    ],
                )

            def qkv_group(b, ot, mt, evac):
                """One (ot, mt) qkv matmul group + evacuation.
                evac: 'act' or 'dve' (engine for the PSUM->SBUF bias evac)."""
                X = Xall[b]
                Qa, Ka, Vf = st[b]["Qa"], st[b]["Ka"], st[b]["Vf"]
                cw = 256 if (b == 0 and mt == 0) else 512
                for off in range(512 * mt, 512 * mt + 512, cw):
                    ms = slice(off, off + cw)
                    pq = pwork.tile([128, cw], F32, name=f"pq{b}{ot}{off}", tag="w")
                    for kt in range(2):
                        nc.tensor.matmul(
                            pq,
                            wqkvT[:, kt, 128 * ot : 128 * ot + 128],
                            X[kt][:, ms],
                            start=(kt == 0),
                            stop=(kt == 1),
                        )
                    dst = [Qa, Ka, Vf[0], Vf[1]][ot]
                    if evac == "act":
                        nc.scalar.activation(
                            dst[:, ms], pq, AF.Identity, bias=bqkv[:, ot : ot + 1]
                        )
                    else:
                        nc.vector.tensor_scalar(
                            dst[:, ms], pq, bqkv[:, ot : ot + 1], None, OP.add
                        )
                    if ot >= 2:
                        ct = ot - 2
                        nr = cw // 32
                        nc.sync.dma_start(
                            Pads[b][ct][
                                :, 1 + off // 32 : 1 + off // 32 + nr, 1:33
                            ],
                            dst[:, ms].rearrange("p (a c) -> p a c", a=nr),
                        )

            def transposes(b, h, evac):
                ct, lo = h // 2, (h % 2) * 64
                par = h % 2
                Vf = st[b]["Vf"]
                pv = pwork.tile([128, 512], BF16, name=f"pv{b}{h}", tag="w")
                for c8 in range(8):
                    nc.tensor.transpose(
                        pv[:, 64 * c8 : 64 * c8 + 64],
                        Vf[ct][lo : lo + 64, 128 * c8 : 128 * c8 + 128],
                        id2[lo : lo + 64, :],
                    )
                dst = Vt[h][:, :, 64 * par : 64 * par + 64]
                pvv = pv.rearrange("p (c d) -> p c d", c=8)
                if evac == "act":
                    nc.scalar.activation(dst, pvv, AF.Identity)
                else:
                    nc.vector.tensor_copy(dst, pvv)

            def dw_taps(b, ct, mt):
                padt = Pads[b][ct]
                dwp = pwork.tile([128, 512], F32, name=f"dw{b}{ct}{mt}", tag="w")
                for tap in range(9):
                    dy, dx = tap // 3, tap % 3
                    r0 = 16 * mt + dy
                    nc.tensor.matmul(
                        dwp,
                        diag[:, 9 * ct + tap, :],
                        padt[:, r0 : r0 + 16, dx : dx + 32],
                        start=(tap == 0),
                        stop=(tap == 8),
                    )
                return dwp

            def dw_add(b, ct, mt, dwp, fine=False):
                out2 = st[b]["out2"]
                steps = ((0, 256), (256, 256)) if fine else ((0, 512),)
                for off, cw in steps:
                    os_ = slice(512 * mt + off, 512 * mt + off + cw)
                    nc.vector.tensor_tensor(
                        out=out2[ct][:, os_],
                        in0=dwp[:, off : off + cw],
                        in1=out2[ct][:, os_],
                        op=OP.add,
                    )

            def pw_unit(b, mt, ot, evac, fine=False):
                out2 = st[b]["out2"]
                steps = ((0, 256), (256, 256)) if fine else ((0, 512),)
                for off, cw in steps:
                    ms = slice(512 * mt + off, 512 * mt + off + cw)
                    pp = pwork.tile(
                        [128, cw], F32, name=f"pp{b}{mt}{ot}{off}", tag="w"
                    )
                    for kt in range(2):
                        nc.tensor.matmul(
                            pp,
                            wpwT[:, kt, 128 * ot : 128 * ot + 128],
                            out2[kt][:, ms],
                            start=(kt == 0),
                            stop=(kt == 1),
                        )
                    osb = outp.tile(
                        [128, cw], BF16, name=f"os{b}{mt}{ot}{off}", tag="os"
                    )
                    if evac == "act":
                        nc.scalar.activation(
                            osb, pp, AF.Identity, bias=bpw[:, ot : ot + 1]
                        )
                    else:
                        nc.vector.tensor_scalar(
                            osb, pp, bpw[:, ot : ot + 1], None, OP.add
                        )
                    [nc.sync, nc.gpsimd][ot].dma_start(
                        out_d.ap()[b, 128 * ot : 128 * ot + 128, ms], osb
                    )

            def attn_unit(b, hp, mt, fillers=(), post=(), fine=False,
                          dve_pairs=DVE_PAIRS):
                """One (hp, mt) attention unit: 4 exp-pairs with PE filler
                woven between the exp emissions and the O-matmuls.  The LAST
                filler must be the unit's dw_taps (its PSUM tile is released
                only by the post-norm dw_add)."""
                Qa, Ka, out2 = st[b]["Qa"], st[b]["Ka"], st[b]["out2"]
                ms = slice(512 * mt, 512 * mt + 512)
                pA = pOp.tile([128, 512], F32, name=f"pa{b}{hp}{mt}", tag="o")
                pB = pOp.tile([128, 512], F32, name=f"pb{b}{hp}{mt}", tag="o")
                banks = [pA, pB]
                fl = list(fillers)
                for p4 in range(4):
                    use_dve = p4 in dve_pairs
                    Et = (et5p if use_dve else et4p).tile(
                        [128, 2, 1024],
                        FP8E5 if use_dve else FP8,
                        name=f"e{b}{hp}{mt}{p4}",
                        tag="e5" if use_dve else "e4",
                    )
                    for half in range(2):
                        lt = 2 * p4 + half
                        ls = slice(128 * lt, 128 * lt + 128)
                        sc = psc.tile(
                            [128, 1024], F32, name=f"sc{b}{hp}{mt}{lt}", tag="sc"
                        )
                        for j in range(2):
                            h = 2 * hp + j
                            nc.tensor.matmul(
                                sc[:, 512 * j : 512 * j + 512],
                                Ka[32 * h : 32 * h + 32, ls],
                                Qa[32 * h : 32 * h + 32, ms],
                                start=True,
                                stop=True,
                                tile_position=(32 * h, 0),
                            )
                        if use_dve:
                            nc.vector.tensor_scalar(
                                Et.bitcast(I8)[:, half, :],
                                sc,
                                float(SCH_A),
                                float(SCH_C),
                                OP.mult,
                                OP.add,
                            )
                        else:
                            nc.scalar.activation(
                                Et[:, half, :], sc, AF.Exp, bias=ebias
                            )
                    if p4 < 3:
                        if fl and len(fl) > 1:
                            fl.pop(0)()
                    else:
                        for f in fl:
                            f()
                        fl = []
                    for j in range(2):
                        h = 2 * hp + j
                        nc.tensor.matmul(
                            banks[j],
                            Vt[h][:, 2 * p4 : 2 * p4 + 2, :],
                            Et[:, :, 512 * j : 512 * j + 512],
                            start=(p4 == 0),
                            stop=(p4 == 3),
                            perf_mode=PM.DoubleRow,
                            skip_group_check=True,
                        )
                # normalize: ZA = [Z_e; Z_o] via shuffles, recip, 2 mults
                steps = ((0, 256), (256, 256)) if fine else ((0, 512),)
                for off, cw in steps:
                    cs = slice(off, off + cw)
                    os_ = slice(512 * mt + off, 512 * mt + off + cw)
                    ZA = smallp.tile(
                        [128, cw], F32, name=f"za{b}{hp}{mt}{off}", tag="za"
                    )
                    nc.vector.stream_shuffle(ZA[0:64, :], pA[64:128, cs], ID32)
                    nc.vector.stream_shuffle(ZA[64:128, :], pB[0:64, cs], ID32)
                    Rz = smallp.tile(
                        [128, cw], F32, name=f"rz{b}{hp}{mt}{off}", tag="rz"
                    )
                    nc.vector.reciprocal_approx_fast(out=Rz, in_=ZA)
                    nc.vector.tensor_tensor(
                        out=out2[hp][0:64, os_],
                        in0=pA[0:64, cs],
                        in1=Rz[0:64, :],
                        op=OP.mult,
                    )
                    nc.vector.tensor_tensor(
                        out=out2[hp][64:128, os_],
                        in0=pB[64:128, cs],
                        in1=Rz[64:128, :],
                        op=OP.mult,
                    )
                for p in post:
                    p()

            def make_dw(b, ct, mt, fine=False):
                state = {}

                def taps():
                    state["dwp"] = dw_taps(b, ct, mt)

                def add():
                    dw_add(b, ct, mt, state["dwp"], fine)

                return taps, add

            assert BL == 2
            init_state(0)
            init_state(1)

            # prologue: b0 K/Q only (evacs: K on ACT, Q on DVE)
            for mt in range(2):
                qkv_group(0, 1, mt, "act")
                qkv_group(0, 0, mt, "dve")
            nc.scalar.dma_start(wpwT, wpwT_d.ap())
            nc.gpsimd.dma_start(bpw, bpw_d.ap())
            nc.gpsimd.dma_start(diag, diag_d.ap())

            # b0 units.  V(0)/transposes(0) are woven in as early fillers
            # (evacs on DVE - ACT is running the exp stream); units whose
            # DVE is evac-heavy run all-ACT exp.
            t00, a00 = make_dw(0, 0, 0)
            attn_unit(0, 0, 0, fillers=[
                lambda: (qkv_group(0, 2, 0, "dve"), qkv_group(0, 2, 1, "dve"),
                         transposes(0, 0, "dve"), transposes(0, 1, "dve")),
                lambda: (qkv_group(0, 3, 0, "dve"), qkv_group(0, 3, 1, "dve")),
                t00,
            ], post=[a00], dve_pairs=())
            t01, a01 = make_dw(0, 0, 1)
            attn_unit(0, 0, 1, fillers=[
                lambda: (transposes(0, 2, "dve"), transposes(0, 3, "dve")),
                lambda: qkv_group(1, 1, 0, "dve"),
                t01,
            ], post=[a01], dve_pairs=())
            t10, a10 = make_dw(0, 1, 0)
            attn_unit(0, 1, 0, fillers=[
                lambda: qkv_group(1, 1, 1, "dve"),
                lambda: (qkv_group(1, 0, 0, "dve"), qkv_group(1, 0, 1, "dve")),
                t10,
            ], post=[a10])
            t11, a11 = make_dw(0, 1, 1)
            attn_unit(0, 1, 1, fillers=[
                lambda: (qkv_group(1, 2, 0, "dve"), qkv_group(1, 2, 1, "dve")),
                lambda: (qkv_group(1, 3, 0, "dve"), qkv_group(1, 3, 1, "dve")),
                t11,
            ], post=[a11])
            # batch-1 vT transposes: must follow ALL batch-0 O-matmuls (Vt
            # tiles are shared), so emit between u3 and u4.
            for h in range(NH):
                transposes(1, h, "dve")

            # b1 units; fillers: b0 pw, then b1 mt=0 pw inside u7
            s00, b00 = make_dw(1, 0, 0)
            attn_unit(1, 0, 0, fillers=[
                lambda: pw_unit(0, 0, 0, "act"),
                lambda: pw_unit(0, 0, 1, "dve"),
                s00,
            ], post=[b00])
            s01, b01 = make_dw(1, 0, 1)
            attn_unit(1, 0, 1, fillers=[
                lambda: pw_unit(0, 1, 0, "act"),
                lambda: pw_unit(0, 1, 1, "dve"),
                s01,
            ], post=[b01])
            s10, b10 = make_dw(1, 1, 0)
            attn_unit(1, 1, 0, fillers=[s10], post=[b10])
            s11, b11 = make_dw(1, 1, 1, fine=True)
            attn_unit(1, 1, 1, fillers=[
                lambda: pw_unit(1, 0, 0, "act"),
                lambda: pw_unit(1, 0, 1, "dve"),
                s11,
            ], post=[b11], fine=True)

            # tail: b1 mt=1 pw
            pw_unit(1, 1, 0, "act", fine=True)
            pw_unit(1, 1, 1, "dve", fine=True)

    nc.compile()
    return nc


def pack_inputs(w_qkv, s_qkv, b_qkv, w_dw, s_dw, b_dw, w_pw, s_pw, b_pw):
    """Host-side weight packing. Returns dict of constant arrays (shared by
    all cores)."""
    import ml_dtypes

    f32 = np.float32
    bf16_ = ml_dtypes.bfloat16
    Wq = (w_qkv[:, :, 0, 0] * s_qkv[:, None]).astype(np.float64)  # [512, 256]
    bq = b_qkv.astype(np.float64).copy()

    # output-channel permutation: [Q_all, K_all, V0, V1]
    perm = []
    for h in range(NH):
        perm += [h * 128 + d for d in range(32)]           # q
    for h in range(NH):
        perm += [h * 128 + 32 + d for d in range(32)]      # k
    for h in range(NH):
        perm += [h * 128 + 64 + d for d in range(64)]      # v
    perm = np.array(perm)
    Wq = Wq[perm]
    bq = bq[perm]
    # fold attention scale into q
    Wq[0:128] *= SCALE
    bq[0:128] *= SCALE

    wqkvT = np.ascontiguousarray(
        Wq.T.reshape(2, 128, CQKV).transpose(1, 0, 2)
    ).astype(bf16_)  # [128, 2, 512]
    bqkv = np.ascontiguousarray(bq.reshape(4, 128).T).astype(f32)  # [128, 4]

    Wp = (w_pw[:, :, 0, 0] * s_pw[:, None]).astype(np.float64)     # [256, 256]
    bp = b_pw.astype(np.float64) + Wp @ b_dw.astype(np.float64)
    wpwT = np.ascontiguousarray(
        Wp.T.reshape(2, 128, CH).transpose(1, 0, 2)
    ).astype(bf16_)  # [128, 2, 256]
    bpw = np.ascontiguousarray(bp.reshape(2, 128).T).astype(f32)   # [128, 2]

    wd = (w_dw[:, 0] * s_dw[:, None, None]).astype(f32)            # [256, 3, 3]
    diag = np.zeros((128, 18, 128), f32)
    for ct in range(2):
        for tap in range(9):
            dy, dx = tap // 3, tap % 3
            idx = np.arange(128)
            diag[idx, 9 * ct + tap, idx] = wd[128 * ct + idx, dy, dx]

    id2 = np.tile(np.eye(64, dtype=f32), (2, 1))                   # [128, 64]

    return {
        "wqkvT": wqkvT,
        "bqkv": bqkv,
        "wpwT": wpwT,
        "bpw": bpw,
        "diag": diag.astype(bf16_),
        "id2": id2.astype(bf16_),
    }


_NC_CACHE = None


def _get_nc():
    global _NC_CACHE
    if _NC_CACHE is None:
        _NC_CACHE = build_bass()
    return _NC_CACHE


def run(inputs, trace=False):
    """Run the bass kernel on 8 cores. inputs = the reference input dict.
    Returns (full_output [16,256,32,32], BassKernelResults)."""
    import ml_dtypes

    x = np.ascontiguousarray(
        np.asarray(inputs["x"], dtype=np.float32).astype(ml_dtypes.bfloat16)
    ).reshape(B, CH, L)
    consts = pack_inputs(
        np.asarray(inputs["w_qkv"], np.float32),
        np.asarray(inputs["s_qkv"], np.float32),
        np.asarray(inputs["b_qkv"], np.float32),
        np.asarray(inputs["w_dw"], np.float32),
        np.asarray(inputs["s_dw"], np.float32),
        np.asarray(inputs["b_dw"], np.float32),
        np.asarray(inputs["w_pw"], np.float32),
        np.asarray(inputs["s_pw"], np.float32),
        np.asarray(inputs["b_pw"], np.float32),
    )
    in_maps = []
    for c in range(NCORES):
        m = dict(consts)
        m["x"] = np.ascontiguousarray(x[c * BL : (c + 1) * BL])
        in_maps.append(m)

    nc = _get_nc()
    res = run_bass_kernel_spmd(
        nc, in_maps, core_ids=list(range(NCORES)), trace=trace
    )
    out = np.concatenate(
        [r["out"].astype(np.float32) for r in res.results], axis=0
    )
    return out.reshape(B, CH, HH, WW), res


def kernel(**inputs) -> np.ndarray:
    out, _ = run(inputs, trace=False)
    return out
